# revision 11
# baseline (speedup 1.0000x reference)
"""Bahdanau-style attention kernel for Trainium2, 8 NeuronCores, data-parallel over
batch, with mask-sparsity: masked positions (mask==1) contribute exactly 0 to the
softmax, so their rows of encoder_outputs are never loaded or computed.

Reference computation, per (b, s):
    energy = tanh(dec @ Wd + enc @ We + b_attn)          # [B,S,H]
    att    = energy @ v_w                                 # [B,S]
    att    = where(mask==1, -1e10, att)
    out    = softmax(att, axis=1)

Full shapes: B=64, S=2048, H=1024. Each core takes 8 batches.

Per-core pipeline (compute in fp16 on the PE, f32 accumulation):
  - live (unmasked) row indices per batch are computed on the host from the mask
    (metadata only); rows are gathered from DRAM by index with dma_gather into
    [128 rows, chunk/128, 1024] tiles, padded to a fixed per-slot count R with
    row 0 (pad results are masked out of Z and never scattered).
  - rows are cast to fp16 and transposed with the xbar DMA transpose so the
    contraction dim (h) lands on partitions.
  - main matmul: psum[kout, rows] += We[h,kout].T @ encT[h,rows] per chunk,
    8 kout-tiles x 8 h-tiles.
  - ACT applies tanh(psum + bias[kout]); bias = dec@Wd + b_attn is per-partition
    (kout on partitions), computed once per batch on the PE.
  - v_w dot is an M=1 matmul over kout partitions -> att scores [1, rows].
  - softmax over live rows only: exp on ACT, multiply by the host-provided
    "j < n_live" mask (kills pads), free-dim reduce for Z, reciprocal, scale.
  - final probs are scattered back to their s positions with gpsimd local_scatter
    (f32 split into two uint16 bitplanes; dead positions stay exactly 0).
"""
import numpy as np

B, S, H = 64, 2048, 1024
NCORES = 8
BPC = B // NCORES          # batches per core
CHUNK = 512                # max rows per chunk
HB = H // 128              # h blocks
KB = H // 128              # kout blocks
R_DEFAULT = 1152           # padded live rows per batch (multiple of 128)
NQ = 4                     # output row quarters for local_scatter (512 f32 each)
QSZ = S // NQ

_graph_cache = {}


def _chunks_of(r):
    out = []
    while r > 0:
        c = min(CHUNK, r)
        out.append(c)
        r -= c
    return out


def _build(R=R_DEFAULT):
    import concourse.bass as bass
    import concourse.bacc as bacc
    import concourse.tile as tile
    from concourse import mybir

    F32 = mybir.dt.float32
    F16 = mybir.dt.float16
    I16 = mybir.dt.int16
    U16 = mybir.dt.uint16
    AF = mybir.ActivationFunctionType
    ALU = mybir.AluOpType

    nc = bacc.Bacc(trn_type="TRN2", target_bir_lowering=False)

    dec_ext = nc.declare_dram_parameter("dec", [BPC, H], F32, isOutput=False)
    enc_ext = nc.declare_dram_parameter("enc", [BPC, S, H], F32, isOutput=False)
    w_ext = nc.declare_dram_parameter("W", [2 * H, H], F32, isOutput=False)
    b_ext = nc.declare_dram_parameter("b", [H], F32, isOutput=False)
    v_ext = nc.declare_dram_parameter("v", [H], F32, isOutput=False)
    gidx_ext = nc.declare_dram_parameter("gidx", [BPC, 128, R // 16], I16, isOutput=False)
    kc_ext = nc.declare_dram_parameter("kc", [BPC, R], F32, isOutput=False)
    sidx_ext = nc.declare_dram_parameter("sidx", [BPC, 16, NQ, 2 * R], I16, isOutput=False)
    out_ext = nc.declare_dram_parameter("out", [BPC, S], F32, isOutput=True)

    chunks = _chunks_of(R)

    with tile.TileContext(nc) as tc:
        with (
            tc.tile_pool(name="weights", bufs=1) as wpool,
            tc.tile_pool(name="consts", bufs=1) as cpool,
            tc.tile_pool(name="wload", bufs=2) as wload,
            tc.tile_pool(name="encload", bufs=2) as epool,
            tc.tile_pool(name="enccast", bufs=2) as bfpool,
            tc.tile_pool(name="enct", bufs=2) as tpool,
            tc.tile_pool(name="energy", bufs=4) as engpool,
            tc.tile_pool(name="rows", bufs=2) as rpool,
            tc.tile_pool(name="psum_mm", bufs=3, space="PSUM") as psum_pool,
            tc.tile_pool(name="psum_vd", bufs=2, space="PSUM") as vd_pool,
        ):
            # ---------------- setup ----------------
            we_f16 = wpool.tile([128, HB, H], F16, tag="we")
            wd_f16 = wpool.tile([128, HB, H], F16, tag="wd")
            for hb in range(HB):
                wt = wload.tile([128, H], F32, tag="wtmp")
                nc.gpsimd.dma_start(out=wt[:], in_=w_ext[H + hb * 128 : H + (hb + 1) * 128, :])
                nc.vector.tensor_copy(we_f16[:, hb, :], wt[:])
                wt2 = wload.tile([128, H], F32, tag="wtmp")
                nc.gpsimd.dma_start(out=wt2[:], in_=w_ext[hb * 128 : (hb + 1) * 128, :])
                nc.vector.tensor_copy(wd_f16[:, hb, :], wt2[:])

            # decT [h -> partitions], per h-block: [128, BPC]
            dect = cpool.tile([128, HB, BPC], F16, tag="dect")
            dtmp = cpool.tile([128, HB, BPC], F32, tag="dectf32")
            for hb in range(HB):
                nc.gpsimd.dma_start(
                    out=dtmp[:, hb, :],
                    in_=dec_ext[:, hb * 128 : (hb + 1) * 128].rearrange("b p -> p b"),
                )
            nc.vector.tensor_copy(dect[:], dtmp[:])

            # b_attn / v_w transposed to [128, KB]
            batt = cpool.tile([128, KB], F32, tag="batt")
            nc.gpsimd.dma_start(out=batt[:], in_=b_ext[:].rearrange("(kb p) -> p kb", p=128))
            vtmpf = cpool.tile([128, KB], F32, tag="vf32")
            nc.gpsimd.dma_start(out=vtmpf[:], in_=v_ext[:].rearrange("(kb p) -> p kb", p=128))
            vt = cpool.tile([128, KB], F16, tag="vt")
            nc.vector.tensor_copy(vt[:], vtmpf[:])

            # bias[kout, b] = (dec @ Wd).T + b_attn, shape [128, KB, BPC]
            bias_sb = cpool.tile([128, KB, BPC], F32, tag="bias")
            for kt in range(KB):
                ps = psum_pool.tile([128, BPC], F32, tag="psetup")
                for hb in range(HB):
                    nc.tensor.matmul(
                        ps[:],
                        wd_f16[:, hb, kt * 128 : (kt + 1) * 128],
                        dect[:, hb, :],
                        start=(hb == 0),
                        stop=(hb == HB - 1),
                    )
                nc.vector.tensor_scalar(bias_sb[:, kt, :], ps[:], batt[:, kt : kt + 1], None, ALU.add)

            # ---------------- main loop ----------------
            for b in range(BPC):
                gidx = rpool.tile([128, R // 16], I16, tag="gidx")
                nc.gpsimd.dma_start(out=gidx[:], in_=gidx_ext[b])
                kc = rpool.tile([1, R], F32, tag="kc")
                nc.gpsimd.dma_start(out=kc[:], in_=kc_ext[b : b + 1, :])
                sidx = rpool.tile([16, NQ, 2 * R], I16, tag="sidx")
                nc.gpsimd.dma_start(out=sidx[:], in_=sidx_ext[b])

                e_comp = rpool.tile([16, R], F32, tag="ecomp")
                zparts = rpool.tile([1, len(chunks)], F32, tag="zparts")
                j0 = 0
                for c, ch in enumerate(chunks):
                    nt = ch // 128
                    enc_f32 = epool.tile([128, nt, H], F32, tag="encf32")
                    nc.gpsimd.dma_gather(
                        out_ap=enc_f32[:],
                        in_ap=enc_ext[b],
                        idxs_ap=gidx[:, j0 // 16 : (j0 + ch) // 16],
                        num_idxs=ch,
                        num_idxs_reg=ch,
                        elem_size=H,
                    )
                    enc_f16 = bfpool.tile([128, nt, H], F16, tag="encf16")
                    nc.vector.tensor_copy(enc_f16[:], enc_f32[:])

                    # xbar transpose each 128-row tile: [128 r, 1024 h] -> [128 h, hb, 128 r]
                    enct = tpool.tile([128, nt, HB, 128], F16, tag="enct")
                    for t in range(nt):
                        nc.sync.dma_start(
                            out=enct[:, t, :, :], in_=enc_f16[:, t, :], transpose=True
                        )

                    vd = vd_pool.tile([1, ch], F32, tag="vdot")
                    pending = []  # staggered vdot emission to keep PE dense
                    for kt in range(KB):
                        pk = psum_pool.tile([128, ch], F32, tag="pmm")
                        for hb in range(HB):
                            nc.tensor.matmul(
                                pk[:],
                                we_f16[:, hb, kt * 128 : (kt + 1) * 128],
                                enct[:, :, hb, :],
                                start=(hb == 0),
                                stop=(hb == HB - 1),
                            )
                        eng = engpool.tile([128, ch], F16, tag="energy")
                        nc.scalar.activation(
                            eng[:], pk[:], AF.Tanh, bias=bias_sb[:, kt, b : b + 1]
                        )
                        pending.append((kt, eng))
                        if len(pending) >= 2:
                            k0, e0 = pending.pop(0)
                            nc.tensor.matmul(
                                vd[:], vt[:, k0 : k0 + 1], e0[:],
                                start=(k0 == 0), stop=(k0 == KB - 1),
                            )
                    for k0, e0 in pending:
                        nc.tensor.matmul(
                            vd[:], vt[:, k0 : k0 + 1], e0[:],
                            start=(k0 == 0), stop=(k0 == KB - 1),
                        )

                    e_raw = rpool.tile([1, CHUNK], F32, tag="eraw")
                    nc.scalar.activation(e_raw[:, :ch], vd[:], AF.Exp)
                    nc.vector.tensor_tensor(
                        e_comp[0:1, j0 : j0 + ch], e_raw[:, :ch], kc[:, j0 : j0 + ch], ALU.mult
                    )
                    nc.vector.tensor_reduce(
                        zparts[:, c : c + 1], e_comp[0:1, j0 : j0 + ch],
                        mybir.AxisListType.XYZW, ALU.add,
                    )
                    j0 += ch

                zacc = rpool.tile([1, 1], F32, tag="zacc")
                nc.vector.tensor_reduce(zacc[:], zparts[:], mybir.AxisListType.XYZW, ALU.add)
                zr = rpool.tile([1, 1], F32, tag="zr")
                nc.vector.reciprocal(zr[:], zacc[:])
                nc.vector.tensor_scalar(e_comp[0:1, :], e_comp[0:1, :], zr[:], None, ALU.mult)

                for q in range(NQ):
                    oq = rpool.tile([16, QSZ], F32, tag="oq")
                    nc.gpsimd.local_scatter(
                        out_ap=oq[:].bitcast(U16),
                        data_ap=e_comp[:].bitcast(U16),
                        idxs_ap=sidx[:, q, :],
                        channels=16,
                        num_elems=2 * QSZ,
                        num_idxs=2 * R,
                    )
                    nc.gpsimd.dma_start(
                        out=out_ext[b : b + 1, q * QSZ : (q + 1) * QSZ], in_=oq[0:1, :]
                    )

    nc.compile()
    return nc


def _get_graph(R=R_DEFAULT):
    if R not in _graph_cache:
        _graph_cache[R] = _build(R)
    return _graph_cache[R]


def _prep_meta(msk):
    """Host-side metadata from the mask: gather indices, pad mask, scatter indices."""
    ncores, bpc, s = NCORES, BPC, S
    counts = (msk == 0).sum(axis=1)
    n_slot = counts.reshape(ncores, bpc).max(axis=0)  # max live rows per slot
    R = max(R_DEFAULT, int(-(-n_slot.max() // 128) * 128))

    gidx = np.zeros((ncores, bpc, 128, R // 16), np.int16)
    kc = np.zeros((ncores, bpc, R), np.float32)
    sidx = np.full((ncores, bpc, 16, NQ, 2 * R), -1, np.int16)
    for ci in range(ncores):
        for b in range(bpc):
            idx = np.where(msk[ci * bpc + b] == 0)[0]
            n = len(idx)
            g = np.zeros(R, np.int64)
            g[:n] = idx
            wrapped = g.reshape(R // 16, 16).T.astype(np.int16)  # [16, R/16]
            gidx[ci, b] = np.tile(wrapped, (8, 1))
            kc[ci, b, :n] = 1.0
            for j in range(n):
                p = idx[j]
                q, pq = divmod(int(p), QSZ)
                sidx[ci, b, 0, q, 2 * j] = 2 * pq
                sidx[ci, b, 0, q, 2 * j + 1] = 2 * pq + 1
    return R, gidx, kc, sidx


def _run(decoder_hidden, encoder_outputs, mask, W_attn, b_attn, v_w, **spmd_kwargs):
    from concourse.bass_utils import run_bass_kernel_spmd

    dec = np.asarray(decoder_hidden, dtype=np.float32)
    enc = np.asarray(encoder_outputs, dtype=np.float32)
    msk = np.asarray(mask, dtype=np.int32)
    W = np.asarray(W_attn, dtype=np.float32)
    bb = np.asarray(b_attn, dtype=np.float32)
    vv = np.asarray(v_w, dtype=np.float32)

    R, gidx, kc, sidx = _prep_meta(msk)
    nc = _get_graph(R)
    in_maps = []
    for i in range(NCORES):
        sl = slice(i * BPC, (i + 1) * BPC)
        in_maps.append(
            {
                "dec": dec[sl],
                "enc": enc[sl],
                "W": W,
                "b": bb,
                "v": vv,
                "gidx": gidx[i],
                "kc": kc[i],
                "sidx": sidx[i],
            }
        )
    res = run_bass_kernel_spmd(nc, in_maps, core_ids=list(range(NCORES)), **spmd_kwargs)
    out = np.concatenate([res.results[i]["out"] for i in range(NCORES)], axis=0)
    return out.astype(np.float32), res


def kernel(decoder_hidden, encoder_outputs, mask, W_attn, b_attn, v_w):
    out, _ = _run(decoder_hidden, encoder_outputs, mask, W_attn, b_attn, v_w)
    return out


# revision 12
# speedup vs baseline: 1.1091x; 1.1091x over previous
"""Bahdanau-style attention kernel for Trainium2, 8 NeuronCores, data-parallel over
batch, with mask-sparsity: masked positions (mask==1) contribute exactly 0 to the
softmax, so their rows of encoder_outputs are never loaded or computed.

Reference computation, per (b, s):
    energy = tanh(dec @ Wd + enc @ We + b_attn)          # [B,S,H]
    att    = energy @ v_w                                 # [B,S]
    att    = where(mask==1, -1e10, att)
    out    = softmax(att, axis=1)

Full shapes: B=64, S=2048, H=1024. Each core takes 8 batches.

Per-core pipeline (compute in fp16 on the PE, f32 accumulation):
  - live (unmasked) row indices per batch are computed on the host from the mask
    (metadata only); rows are gathered from DRAM by index with dma_gather into
    [128 rows, chunk/128, 1024] tiles, padded to a fixed per-slot count R with
    row 0 (pad results are masked out of Z and never scattered).
  - rows are cast to fp16 and transposed with the xbar DMA transpose so the
    contraction dim (h) lands on partitions.
  - main matmul: psum[kout, rows] += We[h,kout].T @ encT[h,rows] per chunk,
    8 kout-tiles x 8 h-tiles.
  - ACT applies tanh(psum + bias[kout]); bias = dec@Wd + b_attn is per-partition
    (kout on partitions), computed once per batch on the PE.
  - v_w dot is an M=1 matmul over kout partitions -> att scores [1, rows].
  - softmax over live rows only: exp on ACT, multiply by the host-provided
    "j < n_live" mask (kills pads), free-dim reduce for Z, reciprocal, scale.
  - final probs are scattered back to their s positions with gpsimd local_scatter
    (f32 split into two uint16 bitplanes; dead positions stay exactly 0).
"""
import numpy as np

B, S, H = 64, 2048, 1024
NCORES = 8
BPC = B // NCORES          # batches per core
CHUNK = 512                # max rows per chunk
HB = H // 128              # h blocks
KB = H // 128              # kout blocks
R_DEFAULT = 1152           # padded live rows per batch (multiple of 128)
NQ = 4                     # output row quarters for local_scatter (512 f32 each)
QSZ = S // NQ

_graph_cache = {}


def _chunks_of(r):
    out = []
    while r > 0:
        c = min(CHUNK, r)
        out.append(c)
        r -= c
    return out


def _build(R=R_DEFAULT):
    import concourse.bass as bass
    import concourse.bacc as bacc
    import concourse.tile as tile
    from concourse import mybir

    F32 = mybir.dt.float32
    F16 = mybir.dt.float16
    I16 = mybir.dt.int16
    U16 = mybir.dt.uint16
    AF = mybir.ActivationFunctionType
    ALU = mybir.AluOpType

    nc = bacc.Bacc(trn_type="TRN2", target_bir_lowering=False)

    dec_ext = nc.declare_dram_parameter("dec", [BPC, H], F32, isOutput=False)
    enc_ext = nc.declare_dram_parameter("enc", [BPC, S, H], F32, isOutput=False)
    w_ext = nc.declare_dram_parameter("W", [2 * H, H], F32, isOutput=False)
    b_ext = nc.declare_dram_parameter("b", [H], F32, isOutput=False)
    v_ext = nc.declare_dram_parameter("v", [H], F32, isOutput=False)
    gidx_ext = nc.declare_dram_parameter("gidx", [BPC, 128, R // 16], I16, isOutput=False)
    kc_ext = nc.declare_dram_parameter("kc", [BPC, R], F32, isOutput=False)
    sidx_ext = nc.declare_dram_parameter("sidx", [BPC, NQ, 2 * R], I16, isOutput=False)
    out_ext = nc.declare_dram_parameter("out", [BPC, S], F32, isOutput=True)

    chunks = _chunks_of(R)

    with tile.TileContext(nc) as tc:
        with (
            tc.tile_pool(name="weights", bufs=1) as wpool,
            tc.tile_pool(name="consts", bufs=1) as cpool,
            tc.tile_pool(name="wload", bufs=2) as wload,
            tc.tile_pool(name="encload", bufs=3) as epool,
            tc.tile_pool(name="enccast", bufs=3) as bfpool,
            tc.tile_pool(name="enct", bufs=3) as tpool,
            tc.tile_pool(name="energy", bufs=4) as engpool,
            tc.tile_pool(name="rows", bufs=2) as rpool,
            tc.tile_pool(name="psum_mm", bufs=3, space="PSUM") as psum_pool,
            tc.tile_pool(name="psum_vd", bufs=2, space="PSUM") as vd_pool,
        ):
            # ---------------- setup ----------------
            we_f16 = wpool.tile([128, HB, H], F16, tag="we")
            wd_f16 = wpool.tile([128, HB, H], F16, tag="wd")
            for hb in range(HB):
                wt = wload.tile([128, H], F32, tag="wtmp")
                nc.scalar.dma_start(out=wt[:], in_=w_ext[H + hb * 128 : H + (hb + 1) * 128, :])
                nc.vector.tensor_copy(we_f16[:, hb, :], wt[:])
                wt2 = wload.tile([128, H], F32, tag="wtmp")
                nc.scalar.dma_start(out=wt2[:], in_=w_ext[hb * 128 : (hb + 1) * 128, :])
                nc.vector.tensor_copy(wd_f16[:, hb, :], wt2[:])

            # decT [h -> partitions], per h-block: [128, BPC]
            dect = cpool.tile([128, HB, BPC], F16, tag="dect")
            dtmp = cpool.tile([128, HB, BPC], F32, tag="dectf32")
            for hb in range(HB):
                nc.scalar.dma_start(
                    out=dtmp[:, hb, :],
                    in_=dec_ext[:, hb * 128 : (hb + 1) * 128].rearrange("b p -> p b"),
                )
            nc.vector.tensor_copy(dect[:], dtmp[:])

            # b_attn / v_w transposed to [128, KB]
            batt = cpool.tile([128, KB], F32, tag="batt")
            nc.scalar.dma_start(out=batt[:], in_=b_ext[:].rearrange("(kb p) -> p kb", p=128))
            vtmpf = cpool.tile([128, KB], F32, tag="vf32")
            nc.scalar.dma_start(out=vtmpf[:], in_=v_ext[:].rearrange("(kb p) -> p kb", p=128))
            vt = cpool.tile([128, KB], F16, tag="vt")
            nc.vector.tensor_copy(vt[:], vtmpf[:])

            # bias[kout, b] = (dec @ Wd).T + b_attn, shape [128, KB, BPC]
            bias_sb = cpool.tile([128, KB, BPC], F32, tag="bias")
            for kt in range(KB):
                ps = psum_pool.tile([128, BPC], F32, tag="psetup")
                for hb in range(HB):
                    nc.tensor.matmul(
                        ps[:],
                        wd_f16[:, hb, kt * 128 : (kt + 1) * 128],
                        dect[:, hb, :],
                        start=(hb == 0),
                        stop=(hb == HB - 1),
                    )
                nc.vector.tensor_scalar(bias_sb[:, kt, :], ps[:], batt[:, kt : kt + 1], None, ALU.add)

            # ---------------- main loop ----------------
            def emit_epilogue(b, e_comp, zparts, sidx_tiles):
                zacc = rpool.tile([1, 1], F32, tag="zacc")
                nc.vector.tensor_reduce(zacc[:], zparts[:], mybir.AxisListType.XYZW, ALU.add)
                zr = rpool.tile([1, 1], F32, tag="zr")
                nc.vector.reciprocal(zr[:], zacc[:])
                nc.vector.tensor_scalar(e_comp[0:1, :], e_comp[0:1, :], zr[:], None, ALU.mult)
                for q in range(NQ):
                    oq = rpool.tile([16, QSZ], F32, tag="oq")
                    nc.gpsimd.local_scatter(
                        out_ap=oq[:].bitcast(U16),
                        data_ap=e_comp[:].bitcast(U16),
                        idxs_ap=sidx_tiles[q][:],
                        channels=16,
                        num_elems=2 * QSZ,
                        num_idxs=2 * R,
                    )
                    nc.scalar.dma_start(
                        out=out_ext[b : b + 1, q * QSZ : (q + 1) * QSZ], in_=oq[0:1, :]
                    )

            pending_epilogue = None
            for b in range(BPC):
                gidx = rpool.tile([128, R // 16], I16, tag="gidx")
                nc.scalar.dma_start(out=gidx[:], in_=gidx_ext[b])
                kc = rpool.tile([1, R], F32, tag="kc")
                nc.scalar.dma_start(out=kc[:], in_=kc_ext[b : b + 1, :])
                sidx_tiles = []
                for q in range(NQ):
                    sq = rpool.tile([16, 2 * R], I16, tag=f"sidx{q}")
                    nc.scalar.dma_start(
                        out=sq[:], in_=sidx_ext[b, q : q + 1, :].broadcast_to([16, 2 * R])
                    )
                    sidx_tiles.append(sq)

                e_comp = rpool.tile([16, R], F32, tag="ecomp")
                zparts = rpool.tile([1, len(chunks)], F32, tag="zparts")
                j0 = 0
                for c, ch in enumerate(chunks):
                    nt = ch // 128
                    enc_f32 = epool.tile([128, nt, H], F32, tag="encf32")
                    nc.gpsimd.dma_gather(
                        out_ap=enc_f32[:],
                        in_ap=enc_ext[b],
                        idxs_ap=gidx[:, j0 // 16 : (j0 + ch) // 16],
                        num_idxs=ch,
                        num_idxs_reg=ch,
                        elem_size=H,
                    )
                    if pending_epilogue is not None and c == 1:
                        emit_epilogue(*pending_epilogue)
                        pending_epilogue = None
                    enc_f16 = bfpool.tile([128, nt, H], F16, tag="encf16")
                    nc.vector.tensor_copy(enc_f16[:], enc_f32[:])

                    # xbar transpose each 128-row tile: [128 r, 1024 h] -> [128 h, hb, 128 r]
                    enct = tpool.tile([128, nt, HB, 128], F16, tag="enct")
                    for t in range(nt):
                        nc.sync.dma_start(
                            out=enct[:, t, :, :], in_=enc_f16[:, t, :], transpose=True
                        )

                    vd = vd_pool.tile([1, ch], F32, tag="vdot")
                    pending = []  # staggered vdot emission to keep PE dense
                    for kt in range(KB):
                        pk = psum_pool.tile([128, ch], F32, tag="pmm")
                        for hb in range(HB):
                            nc.tensor.matmul(
                                pk[:],
                                we_f16[:, hb, kt * 128 : (kt + 1) * 128],
                                enct[:, :, hb, :],
                                start=(hb == 0),
                                stop=(hb == HB - 1),
                            )
                        eng = engpool.tile([128, ch], F16, tag="energy")
                        nc.scalar.activation(
                            eng[:], pk[:], AF.Tanh, bias=bias_sb[:, kt, b : b + 1]
                        )
                        pending.append((kt, eng))
                        if len(pending) >= 2:
                            k0, e0 = pending.pop(0)
                            nc.tensor.matmul(
                                vd[:], vt[:, k0 : k0 + 1], e0[:],
                                start=(k0 == 0), stop=(k0 == KB - 1),
                            )
                    for k0, e0 in pending:
                        nc.tensor.matmul(
                            vd[:], vt[:, k0 : k0 + 1], e0[:],
                            start=(k0 == 0), stop=(k0 == KB - 1),
                        )

                    e_raw = rpool.tile([1, CHUNK], F32, tag="eraw")
                    nc.scalar.activation(e_raw[:, :ch], vd[:], AF.Exp)
                    nc.vector.tensor_tensor(
                        e_comp[0:1, j0 : j0 + ch], e_raw[:, :ch], kc[:, j0 : j0 + ch], ALU.mult
                    )
                    nc.vector.tensor_reduce(
                        zparts[:, c : c + 1], e_comp[0:1, j0 : j0 + ch],
                        mybir.AxisListType.XYZW, ALU.add,
                    )
                    j0 += ch

                pending_epilogue = (b, e_comp, zparts, sidx_tiles)
            emit_epilogue(*pending_epilogue)

    nc.compile()
    return nc


def _get_graph(R=R_DEFAULT):
    if R not in _graph_cache:
        _graph_cache[R] = _build(R)
    return _graph_cache[R]


def _prep_meta(msk):
    """Host-side metadata from the mask: gather indices, pad mask, scatter indices."""
    ncores, bpc, s = NCORES, BPC, S
    counts = (msk == 0).sum(axis=1)
    n_slot = counts.reshape(ncores, bpc).max(axis=0)  # max live rows per slot
    R = max(R_DEFAULT, int(-(-n_slot.max() // 128) * 128))

    gidx = np.zeros((ncores, bpc, 128, R // 16), np.int16)
    kc = np.zeros((ncores, bpc, R), np.float32)
    sidx = np.full((ncores, bpc, NQ, 2 * R), -1, np.int16)
    for ci in range(ncores):
        for b in range(bpc):
            idx = np.where(msk[ci * bpc + b] == 0)[0]
            n = len(idx)
            g = np.zeros(R, np.int64)
            g[:n] = idx
            wrapped = g.reshape(R // 16, 16).T.astype(np.int16)  # [16, R/16]
            gidx[ci, b] = np.tile(wrapped, (8, 1))
            kc[ci, b, :n] = 1.0
            for j in range(n):
                p = idx[j]
                q, pq = divmod(int(p), QSZ)
                sidx[ci, b, q, 2 * j] = 2 * pq
                sidx[ci, b, q, 2 * j + 1] = 2 * pq + 1
    return R, gidx, kc, sidx


def _run(decoder_hidden, encoder_outputs, mask, W_attn, b_attn, v_w, **spmd_kwargs):
    from concourse.bass_utils import run_bass_kernel_spmd

    dec = np.asarray(decoder_hidden, dtype=np.float32)
    enc = np.asarray(encoder_outputs, dtype=np.float32)
    msk = np.asarray(mask, dtype=np.int32)
    W = np.asarray(W_attn, dtype=np.float32)
    bb = np.asarray(b_attn, dtype=np.float32)
    vv = np.asarray(v_w, dtype=np.float32)

    R, gidx, kc, sidx = _prep_meta(msk)
    nc = _get_graph(R)
    in_maps = []
    for i in range(NCORES):
        sl = slice(i * BPC, (i + 1) * BPC)
        in_maps.append(
            {
                "dec": dec[sl],
                "enc": enc[sl],
                "W": W,
                "b": bb,
                "v": vv,
                "gidx": gidx[i],
                "kc": kc[i],
                "sidx": sidx[i],
            }
        )
    res = run_bass_kernel_spmd(nc, in_maps, core_ids=list(range(NCORES)), **spmd_kwargs)
    out = np.concatenate([res.results[i]["out"] for i in range(NCORES)], axis=0)
    return out.astype(np.float32), res


def kernel(decoder_hidden, encoder_outputs, mask, W_attn, b_attn, v_w):
    out, _ = _run(decoder_hidden, encoder_outputs, mask, W_attn, b_attn, v_w)
    return out


# revision 13
# speedup vs baseline: 1.4533x; 1.3104x over previous
"""Bahdanau-style attention kernel for Trainium2, 8 NeuronCores, data-parallel over
batch, with mask-sparsity: masked positions (mask==1) contribute exactly 0 to the
softmax, so their rows of encoder_outputs are never loaded or computed.

Reference computation, per (b, s):
    energy = tanh(dec @ Wd + enc @ We + b_attn)          # [B,S,H]
    att    = energy @ v_w                                 # [B,S]
    att    = where(mask==1, -1e10, att)
    out    = softmax(att, axis=1)

Full shapes: B=64, S=2048, H=1024. Each core takes 8 batches.

Per-core pipeline (compute in fp16 on the PE, f32 accumulation):
  - live (unmasked) row indices per batch are computed on the host from the mask
    (metadata only); rows are gathered from DRAM by index with dma_gather into
    [128 rows, chunk/128, 1024] tiles, padded to a fixed per-slot count R with
    row 0 (pad results are masked out of Z and never scattered).
  - rows are cast to fp16 and transposed with the xbar DMA transpose so the
    contraction dim (h) lands on partitions.
  - main matmul: psum[kout, rows] += We[h,kout].T @ encT[h,rows] per chunk,
    8 kout-tiles x 8 h-tiles.
  - ACT applies tanh(psum + bias[kout]); bias = dec@Wd + b_attn is per-partition
    (kout on partitions), computed once per batch on the PE.
  - v_w dot is an M=1 matmul over kout partitions -> att scores [1, rows].
  - softmax over live rows only: exp on ACT, multiply by the host-provided
    "j < n_live" mask (kills pads), free-dim reduce for Z, reciprocal, scale.
  - final probs are scattered back to their s positions with gpsimd local_scatter
    (f32 split into two uint16 bitplanes; dead positions stay exactly 0).
"""
import numpy as np

B, S, H = 64, 2048, 1024
NCORES = 8
BPC = B // NCORES          # batches per core
CHUNK = 512                # max rows per chunk
HB = H // 128              # h blocks
KB = H // 128              # kout blocks
R_DEFAULT = 1152           # padded live rows per batch (multiple of 128)
NH = 2                     # output row halves for fp16 local_scatter (1024 each)
HSZ = S // NH

_graph_cache = {}


def _chunks_of(r):
    out = []
    while r > 0:
        c = min(CHUNK, r)
        out.append(c)
        r -= c
    return out


def _build(R=R_DEFAULT):
    import concourse.bass as bass
    import concourse.bacc as bacc
    import concourse.tile as tile
    from concourse import mybir

    F32 = mybir.dt.float32
    F16 = mybir.dt.float16
    I16 = mybir.dt.int16
    U16 = mybir.dt.uint16
    AF = mybir.ActivationFunctionType
    ALU = mybir.AluOpType

    nc = bacc.Bacc(trn_type="TRN2", target_bir_lowering=False)

    dec_ext = nc.declare_dram_parameter("dec", [BPC, H], F32, isOutput=False)
    enc_ext = nc.declare_dram_parameter("enc", [BPC, S, H], F32, isOutput=False)
    w_ext = nc.declare_dram_parameter("W", [2 * H, H], F32, isOutput=False)
    b_ext = nc.declare_dram_parameter("b", [H], F32, isOutput=False)
    v_ext = nc.declare_dram_parameter("v", [H], F32, isOutput=False)
    gidx_ext = nc.declare_dram_parameter("gidx", [BPC, 128, R // 16], I16, isOutput=False)
    kc_ext = nc.declare_dram_parameter("kc", [BPC, R], F32, isOutput=False)
    sidx_ext = nc.declare_dram_parameter("sidx", [BPC, NH, R], I16, isOutput=False)
    out_ext = nc.declare_dram_parameter("out", [BPC, S], F32, isOutput=True)

    chunks = _chunks_of(R)

    with tile.TileContext(nc) as tc:
        with (
            tc.tile_pool(name="weights", bufs=1) as wpool,
            tc.tile_pool(name="consts", bufs=1) as cpool,
            tc.tile_pool(name="encload", bufs=4) as epool,
            tc.tile_pool(name="enccast", bufs=3) as bfpool,
            tc.tile_pool(name="enct", bufs=3) as tpool,
            tc.tile_pool(name="energy", bufs=4) as engpool,
            tc.tile_pool(name="rows", bufs=2) as rpool,
            tc.tile_pool(name="psum_mm", bufs=3, space="PSUM") as psum_pool,
            tc.tile_pool(name="psum_vd", bufs=2, space="PSUM") as vd_pool,
        ):
            # ---------------- setup ----------------
            we_f16 = wpool.tile([128, HB, H], F16, tag="we")
            setup_stack = tc.tile_pool(name="wsetup", bufs=1)
            wsetup = setup_stack.__enter__()
            wload_stack = tc.tile_pool(name="wload", bufs=2)
            wload = wload_stack.__enter__()
            wd_f16 = wsetup.tile([128, HB, H], F16, tag="wd")
            for hb in range(HB):
                wt = wload.tile([128, H], F32, tag="wtmp")
                nc.scalar.dma_start(out=wt[:], in_=w_ext[H + hb * 128 : H + (hb + 1) * 128, :])
                nc.vector.tensor_copy(we_f16[:, hb, :], wt[:])
                wt2 = wload.tile([128, H], F32, tag="wtmp")
                nc.scalar.dma_start(out=wt2[:], in_=w_ext[hb * 128 : (hb + 1) * 128, :])
                nc.vector.tensor_copy(wd_f16[:, hb, :], wt2[:])

            # decT [h -> partitions], per h-block: [128, BPC]
            dect = cpool.tile([128, HB, BPC], F16, tag="dect")
            dtmp = cpool.tile([128, HB, BPC], F32, tag="dectf32")
            for hb in range(HB):
                nc.scalar.dma_start(
                    out=dtmp[:, hb, :],
                    in_=dec_ext[:, hb * 128 : (hb + 1) * 128].rearrange("b p -> p b"),
                )
            nc.vector.tensor_copy(dect[:], dtmp[:])

            # b_attn / v_w transposed to [128, KB]
            batt = cpool.tile([128, KB], F32, tag="batt")
            nc.scalar.dma_start(out=batt[:], in_=b_ext[:].rearrange("(kb p) -> p kb", p=128))
            vtmpf = cpool.tile([128, KB], F32, tag="vf32")
            nc.scalar.dma_start(out=vtmpf[:], in_=v_ext[:].rearrange("(kb p) -> p kb", p=128))
            vt = cpool.tile([128, KB], F16, tag="vt")
            nc.vector.tensor_copy(vt[:], vtmpf[:])

            # bias[kout, b] = (dec @ Wd).T + b_attn, shape [128, KB, BPC]
            bias_sb = cpool.tile([128, KB, BPC], F32, tag="bias")
            for kt in range(KB):
                ps = psum_pool.tile([128, BPC], F32, tag="psetup")
                for hb in range(HB):
                    nc.tensor.matmul(
                        ps[:],
                        wd_f16[:, hb, kt * 128 : (kt + 1) * 128],
                        dect[:, hb, :],
                        start=(hb == 0),
                        stop=(hb == HB - 1),
                    )
                nc.vector.tensor_scalar(bias_sb[:, kt, :], ps[:], batt[:, kt : kt + 1], None, ALU.add)
            wload_stack.__exit__(None, None, None)
            setup_stack.__exit__(None, None, None)

            # ---------------- main loop ----------------
            def emit_epilogue(b, e_comp, zparts, sidx_tiles):
                zacc = rpool.tile([1, 1], F32, tag="zacc")
                nc.vector.tensor_reduce(zacc[:], zparts[:], mybir.AxisListType.XYZW, ALU.add)
                zr = rpool.tile([1, 1], F32, tag="zr")
                nc.vector.reciprocal(zr[:], zacc[:])
                e16 = rpool.tile([16, R], F16, tag="e16")
                nc.vector.tensor_scalar(e16[0:1, :], e_comp[0:1, :], zr[:], None, ALU.mult)
                for q in range(NH):
                    oq = rpool.tile([16, HSZ], F16, tag="oq")
                    nc.gpsimd.local_scatter(
                        out_ap=oq[:],
                        data_ap=e16[:],
                        idxs_ap=sidx_tiles[q][:],
                        channels=16,
                        num_elems=HSZ,
                        num_idxs=R,
                    )
                    orow = rpool.tile([1, HSZ], F32, tag="orow")
                    nc.vector.tensor_copy(orow[:], oq[0:1, :])
                    nc.scalar.dma_start(
                        out=out_ext[b : b + 1, q * HSZ : (q + 1) * HSZ], in_=orow[:]
                    )

            pending_epilogue = None
            for b in range(BPC):
                gidx = rpool.tile([128, R // 16], I16, tag="gidx")
                nc.scalar.dma_start(out=gidx[:], in_=gidx_ext[b])
                kc = rpool.tile([1, R], F32, tag="kc")
                nc.scalar.dma_start(out=kc[:], in_=kc_ext[b : b + 1, :])
                sidx_tiles = []
                for q in range(NH):
                    sq = rpool.tile([16, R], I16, tag=f"sidx{q}")
                    nc.scalar.dma_start(
                        out=sq[:], in_=sidx_ext[b, q : q + 1, :].broadcast_to([16, R])
                    )
                    sidx_tiles.append(sq)

                e_comp = rpool.tile([16, R], F32, tag="ecomp")
                zparts = rpool.tile([1, len(chunks)], F32, tag="zparts")
                j0 = 0
                for c, ch in enumerate(chunks):
                    nt = ch // 128
                    enc_f32 = epool.tile([128, nt, H], F32, tag="encf32")
                    nc.gpsimd.dma_gather(
                        out_ap=enc_f32[:],
                        in_ap=enc_ext[b],
                        idxs_ap=gidx[:, j0 // 16 : (j0 + ch) // 16],
                        num_idxs=ch,
                        num_idxs_reg=ch,
                        elem_size=H,
                    )
                    if pending_epilogue is not None and c == len(chunks) - 1:
                        emit_epilogue(*pending_epilogue)
                        pending_epilogue = None
                    enc_f16 = bfpool.tile([128, nt, H], F16, tag="encf16")
                    nc.vector.tensor_copy(enc_f16[:], enc_f32[:])

                    # xbar transpose each 128-row tile: [128 r, 1024 h] -> [128 h, hb, 128 r]
                    enct = tpool.tile([128, nt, HB, 128], F16, tag="enct")
                    for t in range(nt):
                        nc.sync.dma_start(
                            out=enct[:, t, :, :], in_=enc_f16[:, t, :], transpose=True
                        )

                    vd = vd_pool.tile([1, ch], F32, tag="vdot")
                    pending = []  # staggered vdot emission to keep PE dense
                    for kt in range(KB):
                        pk = psum_pool.tile([128, ch], F32, tag="pmm")
                        for hb in range(HB):
                            nc.tensor.matmul(
                                pk[:],
                                we_f16[:, hb, kt * 128 : (kt + 1) * 128],
                                enct[:, :, hb, :],
                                start=(hb == 0),
                                stop=(hb == HB - 1),
                            )
                        eng = engpool.tile([128, ch], F16, tag="energy")
                        nc.scalar.activation(
                            eng[:], pk[:], AF.Tanh, bias=bias_sb[:, kt, b : b + 1]
                        )
                        pending.append((kt, eng))
                        if len(pending) >= 2:
                            k0, e0 = pending.pop(0)
                            nc.tensor.matmul(
                                vd[:], vt[:, k0 : k0 + 1], e0[:],
                                start=(k0 == 0), stop=(k0 == KB - 1),
                            )
                    for k0, e0 in pending:
                        nc.tensor.matmul(
                            vd[:], vt[:, k0 : k0 + 1], e0[:],
                            start=(k0 == 0), stop=(k0 == KB - 1),
                        )

                    e_raw = rpool.tile([1, CHUNK], F32, tag="eraw")
                    nc.scalar.activation(e_raw[:, :ch], vd[:], AF.Exp)
                    nc.vector.tensor_tensor(
                        e_comp[0:1, j0 : j0 + ch], e_raw[:, :ch], kc[:, j0 : j0 + ch], ALU.mult
                    )
                    nc.vector.tensor_reduce(
                        zparts[:, c : c + 1], e_comp[0:1, j0 : j0 + ch],
                        mybir.AxisListType.XYZW, ALU.add,
                    )
                    j0 += ch

                pending_epilogue = (b, e_comp, zparts, sidx_tiles)
            emit_epilogue(*pending_epilogue)

    nc.compile()
    return nc


def _get_graph(R=R_DEFAULT):
    if R not in _graph_cache:
        _graph_cache[R] = _build(R)
    return _graph_cache[R]


def _prep_meta(msk):
    """Host-side metadata from the mask: gather indices, pad mask, scatter indices."""
    ncores, bpc, s = NCORES, BPC, S
    counts = (msk == 0).sum(axis=1)
    n_slot = counts.reshape(ncores, bpc).max(axis=0)  # max live rows per slot
    R = max(R_DEFAULT, int(-(-n_slot.max() // 128) * 128))

    gidx = np.zeros((ncores, bpc, 128, R // 16), np.int16)
    kc = np.zeros((ncores, bpc, R), np.float32)
    sidx = np.full((ncores, bpc, NH, R), -1, np.int16)
    for ci in range(ncores):
        for b in range(bpc):
            idx = np.where(msk[ci * bpc + b] == 0)[0]
            n = len(idx)
            g = np.zeros(R, np.int64)
            g[:n] = idx
            wrapped = g.reshape(R // 16, 16).T.astype(np.int16)  # [16, R/16]
            gidx[ci, b] = np.tile(wrapped, (8, 1))
            kc[ci, b, :n] = 1.0
            for j in range(n):
                q, pq = divmod(int(idx[j]), HSZ)
                sidx[ci, b, q, j] = pq
    return R, gidx, kc, sidx


def _run(decoder_hidden, encoder_outputs, mask, W_attn, b_attn, v_w, **spmd_kwargs):
    from concourse.bass_utils import run_bass_kernel_spmd

    dec = np.asarray(decoder_hidden, dtype=np.float32)
    enc = np.asarray(encoder_outputs, dtype=np.float32)
    msk = np.asarray(mask, dtype=np.int32)
    W = np.asarray(W_attn, dtype=np.float32)
    bb = np.asarray(b_attn, dtype=np.float32)
    vv = np.asarray(v_w, dtype=np.float32)

    R, gidx, kc, sidx = _prep_meta(msk)
    nc = _get_graph(R)
    in_maps = []
    for i in range(NCORES):
        sl = slice(i * BPC, (i + 1) * BPC)
        in_maps.append(
            {
                "dec": dec[sl],
                "enc": enc[sl],
                "W": W,
                "b": bb,
                "v": vv,
                "gidx": gidx[i],
                "kc": kc[i],
                "sidx": sidx[i],
            }
        )
    res = run_bass_kernel_spmd(nc, in_maps, core_ids=list(range(NCORES)), **spmd_kwargs)
    out = np.concatenate([res.results[i]["out"] for i in range(NCORES)], axis=0)
    return out.astype(np.float32), res


def kernel(decoder_hidden, encoder_outputs, mask, W_attn, b_attn, v_w):
    out, _ = _run(decoder_hidden, encoder_outputs, mask, W_attn, b_attn, v_w)
    return out
